# revision 1
# baseline (speedup 1.0000x reference)
"""Multi-head attention (B=2, S=2048, D=1024, H=16, Dk=64) on 8 TRN2 cores.

Sharding: tensor-parallel on heads — 2 heads (dh=128 columns of the QKV
projections) per core.  Each core:
  1. projects qT/kT/vT = (W_slice.T @ x.T) for its 2 heads    [128, 4096]
  2. transposes vT into per-(b,h) [j, d] blocks with an appended
     ones-column (so P@V_aug also yields the softmax row-sums)
  3. scoresT = kT.T-stationary matmul -> pT = exp(scoresT/8) in [j, i]
     layout, PV accumulates oT_aug = [o_unnorm ; rowsums] in PSUM
  4. normalizes via PE-broadcast of 1/rowsum
  5. partialT = Wo_slice.T @ oT                               [1024, 4096]
Host sums the 8 partialT outputs, adds bo, and transposes back.

All matmuls use float32r (full-rate fp32) with fp32 PSUM accumulation.
"""

import numpy as np

D = 1024
NTOK = 4096  # B * S
B = 2
S = 2048
DH = 128  # head-dim block per core (2 heads x 64)
N_CORES = 8

_CACHE = {}


def _build_nc(mm_dtype="float32r"):
    import concourse.bacc as bacc
    import concourse.mybir as mybir
    import concourse.tile as tile

    dt = mybir.dt
    f32 = dt.float32
    mmdt = getattr(dt, mm_dtype)
    AF = mybir.ActivationFunctionType

    def mm(ap):
        return ap

    nc = bacc.Bacc("TRN2", target_bir_lowering=False, debug=False)

    xq = nc.dram_tensor("xq", [D, NTOK], mmdt, kind="ExternalInput").ap()
    xk = nc.dram_tensor("xk", [D, NTOK], mmdt, kind="ExternalInput").ap()
    xv = nc.dram_tensor("xv", [D, NTOK], mmdt, kind="ExternalInput").ap()
    wq = nc.dram_tensor("wq", [128, D], mmdt, kind="ExternalInput").ap()
    wk = nc.dram_tensor("wk", [128, D], mmdt, kind="ExternalInput").ap()
    wv = nc.dram_tensor("wv", [128, D], mmdt, kind="ExternalInput").ap()
    wo = nc.dram_tensor("wo", [128, D], mmdt, kind="ExternalInput").ap()
    bq = nc.dram_tensor("bq", [1, 128], mmdt, kind="ExternalInput").ap()
    bk = nc.dram_tensor("bk", [1, 128], mmdt, kind="ExternalInput").ap()
    bv = nc.dram_tensor("bv", [1, 128], mmdt, kind="ExternalInput").ap()
    c_ident = nc.dram_tensor("c_ident", [128, 64], mmdt, kind="ExternalInput").ap()
    c_ones512 = nc.dram_tensor("c_ones512", [1, 512], mmdt, kind="ExternalInput").ap()
    c_ones64 = nc.dram_tensor("c_ones64", [1, 64], mmdt, kind="ExternalInput").ap()
    pout = nc.dram_tensor("pout", [D, NTOK], f32, kind="ExternalOutput").ap()

    with tile.TileContext(nc) as tc:
        from contextlib import ExitStack

        with ExitStack() as stk:
            const = stk.enter_context(tc.tile_pool(name="const", bufs=1))
            wpool = stk.enter_context(tc.tile_pool(name="w", bufs=1))
            big = stk.enter_context(tc.tile_pool(name="big", bufs=1))
            xpool = stk.enter_context(tc.tile_pool(name="xt", bufs=8))
            ptp = stk.enter_context(tc.tile_pool(name="pt", bufs=4))
            rsp = stk.enter_context(tc.tile_pool(name="rs", bufs=2))
            stp = stk.enter_context(tc.tile_pool(name="st", bufs=4))

            # ---- constants ----
            ident = const.tile([128, 64], mmdt)
            nc.sync.dma_start(out=ident, in_=c_ident)
            ones_row = const.tile([1, 512], mmdt)
            nc.sync.dma_start(out=ones_row, in_=c_ones512)
            ones64 = const.tile([1, 64], mmdt)
            nc.sync.dma_start(out=ones64, in_=c_ones64)

            # ---- weights / biases ----
            wq_sb = wpool.tile([128, D], mmdt)
            wk_sb = wpool.tile([128, D], mmdt)
            wv_sb = wpool.tile([128, D], mmdt)
            wo_sb = wpool.tile([128, D], mmdt)
            nc.sync.dma_start(out=wq_sb, in_=wq)
            nc.sync.dma_start(out=wk_sb, in_=wk)
            nc.sync.dma_start(out=wv_sb, in_=wv)
            nc.sync.dma_start(out=wo_sb, in_=wo)
            bq_sb = const.tile([1, 128], mmdt)
            bk_sb = const.tile([1, 128], mmdt)
            bv_sb = const.tile([1, 128], mmdt)
            nc.sync.dma_start(out=bq_sb, in_=bq)
            nc.sync.dma_start(out=bk_sb, in_=bk)
            nc.sync.dma_start(out=bv_sb, in_=bv)

            # ---- persistent activations ----
            qT = big.tile([128, NTOK], mmdt)  # [dh, tok]
            kT = big.tile([128, NTOK], mmdt)
            vT = big.tile([128, NTOK], mmdt)
            v_sb = big.tile([128, 4 * 16 * 65], mmdt)  # [j, (b,h)*jt*(64+1)]
            oT = big.tile([128, NTOK], mmdt)  # [dh, tok] normalized context

            # ones columns: memset whole tile; v blocks overwritten by transposes
            nc.vector.memset(v_sb, 1.0)
            v_r = v_sb.rearrange("p (t c) -> p t c", c=65)

            def emit_proj(b):
                """projections qT/kT/vT for batch b (cols b*2048..)"""
                with tc.tile_pool(name=f"pp{b}", bufs=8, space="PSUM") as pp:
                    for x_dram, w_sb, b_sb, dst, pnm in (
                        (xq, wq_sb, bq_sb, qT, "q"),
                        (xk, wk_sb, bk_sb, kT, "k"),
                        (xv, wv_sb, bv_sb, vT, "v"),
                    ):
                        acc = [
                            pp.tile([128, 512], f32, tag="pp", name=f"acc{pnm}{b}_{nn}")
                            for nn in range(4)
                        ]
                        for kk in range(8):
                            x_t = xpool.tile([128, 2048], mmdt, tag="xt", name=f"x{pnm}{b}_{kk}")
                            nc.sync.dma_start(
                                out=x_t,
                                in_=x_dram[
                                    kk * 128 : (kk + 1) * 128,
                                    b * 2048 : (b + 1) * 2048,
                                ],
                            )
                            for nn in range(4):
                                nc.tensor.matmul(
                                    acc[nn],
                                    lhsT=w_sb[:, kk * 128 : (kk + 1) * 128],
                                    rhs=x_t[:, nn * 512 : (nn + 1) * 512],
                                    start=(kk == 0),
                                    stop=False,
                                )
                        for nn in range(4):
                            # bias: acc += b_sb.T @ ones  (adds bias to each col)
                            nc.tensor.matmul(
                                acc[nn], lhsT=b_sb, rhs=ones_row, start=False, stop=True
                            )
                        for nn in range(4):
                            col = b * 2048 + nn * 512
                            eng = nc.scalar.copy if nn % 2 == 0 else nc.vector.tensor_copy
                            eng(dst[:, col : col + 512], acc[nn])

            def emit_transp(b):
                """vT -> v_sb [j, d] blocks for batch b"""
                with tc.tile_pool(name=f"tp{b}", bufs=3, space="PSUM") as tpp:
                    for h in range(2):
                        bh = b * 2 + h
                        for g in range(4):  # groups of 4 j-tiles
                            tp = tpp.tile([128, 4 * 64], mmdt, tag="tp", name=f"tp{bh}_{g}")
                            for u in range(4):
                                jb = g * 4 + u
                                nc.tensor.transpose(
                                    tp[:, u * 64 : (u + 1) * 64],
                                    vT[
                                        h * 64 : (h + 1) * 64,
                                        b * 2048 + jb * 128 : b * 2048 + (jb + 1) * 128,
                                    ],
                                    ident[h * 64 : (h + 1) * 64, :],
                                )
                            tp_r = tp.rearrange("p (t c) -> p t c", c=64)
                            nc.scalar.copy(
                                v_r[:, bh * 16 + g * 4 : bh * 16 + g * 4 + 4, 0:64],
                                tp_r,
                            )

            def emit_outproj(b, evac_eng):
                """partialT[:, b cols] = Wo_c.T @ oT ; borrows caller's psum pool"""
                for c4 in range(4):
                    c8 = b * 4 + c4
                    for dt_ in range(8):
                        op = opj_pool[0].tile(
                            [128, 512], f32, tag=opj_pool[1], name=f"op{c8}_{dt_}"
                        )
                        nc.tensor.matmul(
                            op,
                            lhsT=wo_sb[:, dt_ * 128 : (dt_ + 1) * 128],
                            rhs=oT[:, c8 * 512 : (c8 + 1) * 512],
                            start=True,
                            stop=True,
                        )
                        st = stp.tile([128, 512], f32, tag="st", name=f"st{c8}_{dt_}")
                        eng = (
                            nc.vector.tensor_copy
                            if evac_eng == "dve" or dt_ % 2
                            else nc.scalar.copy
                        )
                        eng(st, op)
                        nc.sync.dma_start(
                            out=pout[
                                dt_ * 128 : (dt_ + 1) * 128,
                                c8 * 512 : (c8 + 1) * 512,
                            ],
                            in_=st,
                        )

            # ---- attention passes with deferred finalize ----
            def emit_pass(scp, opp, rpp, b, h, half, pending):
                bh = b * 2 + h
                i0 = b * 2048 + half * 1024
                o_ps = opp.tile([65, 1024], f32, tag="ops", name=f"o{bh}_{half}")
                for jt in range(16):
                    for c in range(2):
                        sc = scp.tile([128, 512], f32, tag="sc", name=f"s{bh}_{half}_{jt}_{c}")
                        nc.tensor.matmul(
                            sc,
                            lhsT=kT[
                                h * 64 : (h + 1) * 64,
                                b * 2048 + jt * 128 : b * 2048 + (jt + 1) * 128,
                            ],
                            rhs=qT[h * 64 : (h + 1) * 64, i0 + c * 512 : i0 + (c + 1) * 512],
                            start=True,
                            stop=True,
                        )
                        pt = ptp.tile([128, 512], mmdt, tag="pt", name=f"p{bh}_{half}_{jt}_{c}")
                        nc.scalar.activation(pt, sc, AF.Exp, scale=0.125)
                        nc.tensor.matmul(
                            o_ps[:, c * 512 : (c + 1) * 512],
                            lhsT=v_sb[:, (bh * 16 + jt) * 65 : (bh * 16 + jt + 1) * 65],
                            rhs=pt,
                            start=(jt == 0),
                            stop=(jt == 15),
                        )
                    if jt == 2 and pending is not None:
                        emit_finalize(rpp, *pending)
                        pending = None
                return (o_ps, b, h, half)

            def emit_finalize(rpp, o_ps, b, h, half):
                """normalize: oT[h cols] = o_unnorm * broadcast(1/rowsum)"""
                bh = b * 2 + h
                i0 = b * 2048 + half * 1024
                rinv = rsp.tile([1, 1024], mmdt, tag="rinv", name=f"ri{bh}_{half}")
                with nc.allow_low_precision(reason="fp16 rinv is plenty"):
                    nc.vector.reciprocal(rinv, o_ps[64:65, :])
                Rp = rpp.tile([64, 1024], f32, tag="rp", name=f"R{bh}_{half}")
                for c in range(2):
                    nc.tensor.matmul(
                        Rp[:, c * 512 : (c + 1) * 512],
                        lhsT=ones64,
                        rhs=rinv[:, c * 512 : (c + 1) * 512],
                        start=True,
                        stop=True,
                    )
                Rs = rsp.tile([64, 1024], f32, tag="rs", name=f"Rs{bh}_{half}")
                nc.vector.tensor_copy(Rs, Rp)
                nc.vector.tensor_mul(
                    oT[h * 64 : (h + 1) * 64, i0 : i0 + 1024], o_ps[0:64, :], Rs
                )

            # =========== emission schedule ===========
            emit_proj(0)
            emit_transp(0)
            pending = None
            with (
                tc.tile_pool(name="scA", bufs=2, space="PSUM") as scA,
                tc.tile_pool(name="opsA", bufs=2, space="PSUM") as opsA,
                tc.tile_pool(name="rpA", bufs=1, space="PSUM") as rpA,
            ):
                for h in range(2):
                    for half in range(2):
                        pending = emit_pass(scA, opsA, rpA, 0, h, half, pending)
                emit_finalize(rpA, *pending)
                pending = None

            emit_proj(1)
            emit_transp(1)
            with (
                tc.tile_pool(name="scB", bufs=2, space="PSUM") as scB,
                tc.tile_pool(name="opsB", bufs=2, space="PSUM") as opsB,
                tc.tile_pool(name="rpB", bufs=1, space="PSUM") as rpB,
            ):
                pending = emit_pass(scB, opsB, rpB, 1, 0, 0, pending)
                pending = emit_pass(scB, opsB, rpB, 1, 0, 1, pending)
                # b0 out-projection overlaps b1 attention (borrows scB slots)
                opj_pool = (scB, "sc")
                emit_outproj(0, "dve")
                pending = emit_pass(scB, opsB, rpB, 1, 1, 0, pending)
                pending = emit_pass(scB, opsB, rpB, 1, 1, 1, pending)
                emit_finalize(rpB, *pending)
                opj_pool = (scB, "sc")
                emit_outproj(1, "mix")

    nc.compile()
    return nc


MM_DTYPE = "float16"


def _get_nc():
    key = ("nc", MM_DTYPE)
    if key not in _CACHE:
        _CACHE[key] = _build_nc(MM_DTYPE)
    return _CACHE[key]


def _ensure_ntff_hook():
    """Register the NTFF profile hook module if the image lacks it."""
    import sys
    import types

    if "antenv.axon_hooks" in sys.modules:
        return
    try:
        from trn_agent_boot.trn_boot import _ntff_profile_via_ctypes
    except Exception:
        return
    hook = None
    try:
        hook = _ntff_profile_via_ctypes("/opt/axon/libaxon_pjrt.so")
    except Exception:
        hook = None
    mod = types.ModuleType("antenv.axon_hooks")
    mod._hook = hook
    mod.get_axon_ntff_profile_hook = lambda: mod._hook
    mod.set_axon_ntff_profile_hook = lambda h: setattr(mod, "_hook", h)
    sys.modules["antenv.axon_hooks"] = mod


def _run(inputs, trace=False):
    from concourse import bass_utils

    if trace:
        _ensure_ntff_hook()

    nc = _get_nc()
    query = np.asarray(inputs["query"], np.float32)
    key = np.asarray(inputs["key"], np.float32)
    value = np.asarray(inputs["value"], np.float32)
    Wq = np.asarray(inputs["Wq"], np.float32)
    Wk = np.asarray(inputs["Wk"], np.float32)
    Wv = np.asarray(inputs["Wv"], np.float32)
    Wo = np.asarray(inputs["Wo"], np.float32)
    bq = np.asarray(inputs["bq"], np.float32)
    bk = np.asarray(inputs["bk"], np.float32)
    bv = np.asarray(inputs["bv"], np.float32)
    bo = np.asarray(inputs["bo"], np.float32)

    if MM_DTYPE == "bfloat16":
        import ml_dtypes

        ext_dt = ml_dtypes.bfloat16
    elif MM_DTYPE == "float16":
        ext_dt = np.float16
    else:
        ext_dt = np.float32

    xqT = np.ascontiguousarray(query.reshape(NTOK, D).T.astype(ext_dt))
    xkT = np.ascontiguousarray(key.reshape(NTOK, D).T.astype(ext_dt))
    xvT = np.ascontiguousarray(value.reshape(NTOK, D).T.astype(ext_dt))

    def pack_w(Wc):
        return np.ascontiguousarray(
            Wc.reshape(8, 128, 128).transpose(1, 0, 2).reshape(128, D).astype(ext_dt)
        )

    ident_np = np.zeros((128, 64), np.float32)
    ident_np[np.arange(64), np.arange(64)] = 1.0
    ident_np[64 + np.arange(64), np.arange(64)] = 1.0
    consts = {
        "c_ident": np.ascontiguousarray(ident_np.astype(ext_dt)),
        "c_ones512": np.ones((1, 512), ext_dt),
        "c_ones64": np.ones((1, 64), ext_dt),
    }
    in_maps = []
    for c in range(N_CORES):
        sl = slice(c * 128, (c + 1) * 128)
        in_maps.append(
            {
                **consts,
                "xq": xqT,
                "xk": xkT,
                "xv": xvT,
                "wq": pack_w(Wq[:, sl]),
                "wk": pack_w(Wk[:, sl]),
                "wv": pack_w(Wv[:, sl]),
                "wo": np.ascontiguousarray(Wo[sl, :].astype(ext_dt)),
                "bq": np.ascontiguousarray(bq[sl].reshape(1, 128).astype(ext_dt)),
                "bk": np.ascontiguousarray(bk[sl].reshape(1, 128).astype(ext_dt)),
                "bv": np.ascontiguousarray(bv[sl].reshape(1, 128).astype(ext_dt)),
            }
        )

    res = bass_utils.run_bass_kernel_spmd(
        nc, in_maps, core_ids=list(range(N_CORES)), trace=trace
    )
    outT = np.zeros((D, NTOK), np.float64)
    for c in range(N_CORES):
        outT += np.asarray(res.results[c]["pout"], np.float64)
    out = (outT.T + bo.astype(np.float64)).astype(np.float32)
    return out.reshape(B, S, D), res


def kernel(**inputs):
    out, _ = _run(inputs, trace=False)
    return out



# revision 15
# speedup vs baseline: 1.9167x; 1.9167x over previous
"""Multi-head attention (B=2, S=2048, D=1024, H=16, Dk=64) on 8 TRN2 cores.

Sharding: tensor-parallel on heads - 2 heads (dh=128 columns of the QKV
projections) per core.  All matmuls fp16 (fp8 fails the 2e-2 gate: attention
is concentrated, so v/p quantization error does not average out).

Design targets a gapless PE pipeline (keeps the 2.4GHz p-state) with the
Scalar engine doing nothing but Exp (no activation-table thrash):
  - per (b, h, i-quarter): 8 iterations of [2 score matmuls -> one
    [128,1024] exp -> 2 PV matmuls], software-pipelined so PV for tile t
    issues after the scores of tile t+1 (exp latency hidden).
  - QKV bias folded into the PSUM->SBUF evacuation (DVE tensor_scalar).
  - softmax normalization: ones-column rowsums, DVE copy to SBUF,
    reciprocal_approx_fast (DVE), fp16 PE broadcast matmul, DVE multiply.
  - b1 projections and out-projection chunks are woven into the PE stream
    inside the attention loops so the PE never idles.
  - partial outputs written fp16; host sums the 8 partials in fp32.
"""

import numpy as np

D = 1024
NTOK = 4096  # B * S
B = 2
S = 2048
N_CORES = 8

_CACHE = {}


def _build_nc():
    import concourse.bacc as bacc
    import concourse.mybir as mybir
    import concourse.tile as tile

    dt = mybir.dt
    f32 = dt.float32
    f16 = dt.float16
    AF = mybir.ActivationFunctionType

    nc = bacc.Bacc("TRN2", target_bir_lowering=False, debug=False)

    xq = nc.dram_tensor("xq", [D, NTOK], f16, kind="ExternalInput").ap()
    xk = nc.dram_tensor("xk", [D, NTOK], f16, kind="ExternalInput").ap()
    xv = nc.dram_tensor("xv", [D, NTOK], f16, kind="ExternalInput").ap()
    wq = nc.dram_tensor("wq", [128, D], f16, kind="ExternalInput").ap()
    wk = nc.dram_tensor("wk", [128, D], f16, kind="ExternalInput").ap()
    wv = nc.dram_tensor("wv", [128, D], f16, kind="ExternalInput").ap()
    wo = nc.dram_tensor("wo", [128, D], f16, kind="ExternalInput").ap()
    bq = nc.dram_tensor("bq", [128, 1], f16, kind="ExternalInput").ap()
    bk = nc.dram_tensor("bk", [128, 1], f16, kind="ExternalInput").ap()
    bv = nc.dram_tensor("bv", [128, 1], f16, kind="ExternalInput").ap()
    c_ident = nc.dram_tensor("c_ident", [128, 64], f16, kind="ExternalInput").ap()
    c_ones64 = nc.dram_tensor("c_ones64", [1, 64], f16, kind="ExternalInput").ap()
    pout = nc.dram_tensor("pout", [D, NTOK], f16, kind="ExternalOutput").ap()

    with tile.TileContext(nc) as tc:
        from contextlib import ExitStack

        with ExitStack() as stk:
            const = stk.enter_context(tc.tile_pool(name="const", bufs=1))
            big = stk.enter_context(tc.tile_pool(name="big", bufs=1))
            xp = stk.enter_context(tc.tile_pool(name="xp", bufs=24))
            ptp = stk.enter_context(tc.tile_pool(name="ptp", bufs=3))
            stp = stk.enter_context(tc.tile_pool(name="stp", bufs=4))
            rsp = stk.enter_context(tc.tile_pool(name="rsp", bufs=4))
            # PSUM: work 2x[128,512]f32 (2 banks) + sc 2x[128,1024]f32
            # (4 banks) + ops 2x[65,512]f32 (2 banks) = 8 banks exactly.
            work = stk.enter_context(tc.tile_pool(name="work", bufs=2, space="PSUM"))
            scp = stk.enter_context(tc.tile_pool(name="scp", bufs=2, space="PSUM"))
            opp = stk.enter_context(tc.tile_pool(name="opp", bufs=2, space="PSUM"))

            # ---- constants / weights ----
            ident = const.tile([128, 64], f16)
            nc.sync.dma_start(out=ident, in_=c_ident)
            ones64 = const.tile([1, 64], f16)
            nc.sync.dma_start(out=ones64, in_=c_ones64)
            w_sb = {}
            b_sb = {}
            for nm, wdr, bdr in (("q", wq, bq), ("k", wk, bk), ("v", wv, bv)):
                w_sb[nm] = const.tile([128, D], f16, name=f"w{nm}")
                nc.sync.dma_start(out=w_sb[nm], in_=wdr)
                b_sb[nm] = const.tile([128, 1], f16, name=f"b{nm}")
                nc.sync.dma_start(out=b_sb[nm], in_=bdr)
            wo_sb = const.tile([128, D], f16, name="wo")
            nc.sync.dma_start(out=wo_sb, in_=wo)

            # ---- persistent activations ----
            qT = big.tile([128, NTOK], f16)  # [dh, tok]
            kT = big.tile([128, NTOK], f16)
            vT = big.tile([128, NTOK], f16)
            oT = big.tile([128, NTOK], f16)  # normalized context, [dh, tok]
            v_sb = big.tile([128, 4 * 16 * 65], f16)  # [j, (b,h)*jt*(64+1)]
            nc.vector.memset(v_sb, 1.0)
            v_r = v_sb.rearrange("p (t c) -> p t c", c=65)

            dst_of = {"q": qT, "k": kT, "v": vT}
            x_of = {"q": xq, "k": xk, "v": xv}

            # ---- x staging: issue DMAs for a (tensor, batch) group ----
            x_tiles = {}

            def emit_x_dma(nm, b):
                for kk in range(8):
                    t = xp.tile([128, 2048], f16, tag="xt", name=f"x{nm}{b}_{kk}")
                    nc.sync.dma_start(
                        out=t,
                        in_=x_of[nm][kk * 128 : (kk + 1) * 128, b * 2048 : (b + 1) * 2048],
                    )
                    x_tiles[(nm, b, kk)] = t

            # ---- generators: each yield is roughly one PE quantum ----
            def gen_proj(b, parts=(("k", (0, 1, 2, 3)), ("v", (0, 1, 2, 3)), ("q", (0, 1, 2, 3)))):
                """q/k/v projections + v transposes for batch b."""
                for nm, nns in parts:
                    for nn in nns:
                        acc = work.tile([128, 512], f32, tag="wk", name=f"a{nm}{b}{nn}")
                        for kk in range(8):
                            nc.tensor.matmul(
                                acc,
                                lhsT=w_sb[nm][:, kk * 128 : (kk + 1) * 128],
                                rhs=x_tiles[(nm, b, kk)][:, nn * 512 : (nn + 1) * 512],
                                start=(kk == 0),
                                stop=(kk == 7),
                            )
                            if kk == 3:
                                yield
                        col = b * 2048 + nn * 512
                        nc.vector.tensor_scalar_add(
                            dst_of[nm][:, col : col + 512], acc, b_sb[nm]
                        )
                        yield

            def gen_vtransp(b):
                """vT -> v_sb [j, d] blocks; woven into attention as bg."""
                for h in range(2):
                    bh = b * 2 + h
                    for g in range(4):
                        tp = work.tile([128, 256], f16, tag="wk", name=f"t{bh}{g}")
                        for u in range(4):
                            jb = g * 4 + u
                            nc.tensor.transpose(
                                tp[:, u * 64 : (u + 1) * 64],
                                vT[
                                    h * 64 : (h + 1) * 64,
                                    b * 2048 + jb * 128 : b * 2048 + (jb + 1) * 128,
                                ],
                                ident[h * 64 : (h + 1) * 64, :],
                            )
                        tp_r = tp.rearrange("p (t c) -> p t c", c=64)
                        nc.vector.tensor_copy(
                            v_r[:, bh * 16 + g * 4 : bh * 16 + g * 4 + 4, 0:64],
                            tp_r,
                        )
                        yield

            def gen_outproj(b, c):
                """partialT[:, 512-token chunk] = Wo_c.T @ oT ; fp16 out."""
                c8 = b * 4 + c
                for dt_ in range(8):
                    op = work.tile([128, 512], f32, tag="wk", name=f"po{c8}_{dt_}")
                    nc.tensor.matmul(
                        op,
                        lhsT=wo_sb[:, dt_ * 128 : (dt_ + 1) * 128],
                        rhs=oT[:, c8 * 512 : (c8 + 1) * 512],
                        start=True,
                        stop=True,
                    )
                    st = stp.tile([128, 512], f16, tag="st", name=f"s{c8}_{dt_}")
                    nc.vector.tensor_copy(st, op)
                    nc.sync.dma_start(
                        out=pout[dt_ * 128 : (dt_ + 1) * 128, c8 * 512 : (c8 + 1) * 512],
                        in_=st,
                    )
                    yield

            pend_fin = []  # deferred finalize state: (o_ps, rs16, b, h, c)

            def emit_finalize_front(o_ps, b, h, c):
                """rinv = exp(-ln(rowsum)): DVE copy to partition 0, then two
                Scalar-engine ops (ln/exp share one activation table, so no
                table reloads).  The PE broadcast matmul is deferred so the
                PE queue never waits on this chain."""
                bh = b * 2 + h
                rs32 = rsp.tile([1, 512], f32, tag="r32", name=f"r{bh}{c}")
                nc.vector.tensor_copy(rs32, o_ps[64:65, :])
                ri32 = rsp.tile([1, 512], f32, tag="ri32", name=f"rr{bh}{c}")
                nc.vector.reciprocal_approx_fast(ri32, rs32)
                rs16 = rsp.tile([1, 512], f16, tag="r16", name=f"ri{bh}{c}")
                nc.vector.tensor_copy(rs16, ri32)
                pend_fin.append((o_ps, rs16, b, h, c))

            def emit_finalize_back():
                if not pend_fin:
                    return
                o_ps, rs16, b, h, c = pend_fin.pop(0)
                bh = b * 2 + h
                R = work.tile([64, 512], f32, tag="wk", name=f"R{bh}{c}")
                nc.tensor.matmul(R, lhsT=ones64, rhs=rs16, start=True, stop=True)
                Rs = rsp.tile([64, 512], f32, tag="Rs", name=f"Rs{bh}{c}")
                nc.vector.tensor_copy(Rs, R)
                nc.vector.tensor_mul(
                    oT[h * 64 : (h + 1) * 64, b * 2048 + c * 512 : b * 2048 + (c + 1) * 512],
                    o_ps[0:64, :],
                    Rs,
                )
                on_block_done(b, h, c)

            def gen_attention():
                """Both batches, blocks (b, c, h); software-pipelined PV and
                deferred finalize."""
                pend = None  # (ptt, o_ps, b, h, c, t)
                for b in range(B):
                    for c in range(4):
                        for h in range(2):
                            bh = b * 2 + h
                            i0 = b * 2048 + c * 512
                            o_ps = opp.tile(
                                [65, 512], f32, tag="ops", name=f"o{bh}_{c}"
                            )
                            for t in range(8):
                                sct = scp.tile(
                                    [128, 1024], f32, tag="sc", name=f"s{bh}{c}{t}"
                                )
                                for u in (0, 1):
                                    jt = 2 * t + u
                                    nc.tensor.matmul(
                                        sct[:, u * 512 : (u + 1) * 512],
                                        lhsT=kT[
                                            h * 64 : (h + 1) * 64,
                                            b * 2048 + jt * 128 : b * 2048 + (jt + 1) * 128,
                                        ],
                                        rhs=qT[h * 64 : (h + 1) * 64, i0 : i0 + 512],
                                        start=True,
                                        stop=True,
                                    )
                                ptt = ptp.tile(
                                    [128, 1024], f16, tag="pt", name=f"p{bh}{c}{t}"
                                )
                                nc.scalar.activation(ptt, sct, AF.Exp, scale=0.125)
                                if pend is not None:
                                    emit_pv(*pend)
                                pend = (ptt, o_ps, b, h, c, t)
                                if t == 5:
                                    emit_finalize_back()
                                yield
                # drain last tile + remaining finalizes
                emit_pv(*pend)
                while pend_fin:
                    emit_finalize_back()

            def emit_pv(ptt, o_ps, b, h, c, t):
                bh = b * 2 + h
                for u in (0, 1):
                    jt = 2 * t + u
                    nc.tensor.matmul(
                        o_ps,
                        lhsT=v_sb[:, (bh * 16 + jt) * 65 : (bh * 16 + jt) * 65 + 65],
                        rhs=ptt[:, u * 512 : (u + 1) * 512],
                        start=(jt == 0),
                        stop=(jt == 15),
                    )
                if t == 7:
                    emit_finalize_front(o_ps, b, h, c)

            # =========== emission schedule ===========
            from collections import deque

            bg = deque()
            done_blocks = set()

            def on_block_done(b, h, c):
                done_blocks.add((b, h, c))
                if (b, 0, c) in done_blocks and (b, 1, c) in done_blocks:
                    bg.append(gen_outproj(b, c))

            # startup: only b0 x DMAs (b1's would steal startup bandwidth)
            for nm in ("k", "v", "q"):
                emit_x_dma(nm, 0)

            # b0 k/v projections + q first quarter run solid (transposes deferred)
            for _ in gen_proj(0, (("k", (0, 1, 2, 3)), ("v", (0, 1, 2, 3)), ("q", (0,)))):
                pass
            # b1 x DMAs start now: b0 startup traffic is done
            for nm in ("k", "v", "q"):
                emit_x_dma(nm, 1)
            # b0 v transposes + rest of b0 q projection are the first bg work
            bg.append(gen_vtransp(0))
            bg.append(gen_proj(0, (("q", (1, 2, 3)),)))

            def advance(gen):
                """Run one unit of a generator; True if it emitted, False if done."""
                try:
                    next(gen)
                    return True
                except StopIteration:
                    return False

            # attention over both batches; weave bg work into the PE stream.
            # b1 projection weave starts ~16 yields in (its x DMAs land late).
            from itertools import chain as _chain

            attn = gen_attention()
            proj1 = _chain(
                gen_proj(1, (("k", (0, 1, 2, 3)), ("v", (0, 1, 2, 3)))),
                gen_vtransp(1),
                gen_proj(1, (("q", (0, 1, 2, 3)),)),
            )
            proj1_active = True
            for i, _ in enumerate(attn):
                if i >= 60 and proj1_active:
                    # b1 attention imminent: drain remaining b1 projection
                    while advance(proj1):
                        pass
                    proj1_active = False
                if i >= 16 and proj1_active:
                    if advance(proj1):
                        continue
                    proj1_active = False
                if bg and not advance(bg[0]):
                    bg.popleft()
            # drain remaining background (last out-projection chunks)
            while bg:
                if not advance(bg[0]):
                    bg.popleft()

    nc.compile()
    return nc


def _get_nc():
    if "nc" not in _CACHE:
        _CACHE["nc"] = _build_nc()
    return _CACHE["nc"]


def _ensure_ntff_hook():
    """Register the NTFF profile hook module if the image lacks it."""
    import sys
    import types

    if "antenv.axon_hooks" in sys.modules:
        return
    try:
        from trn_agent_boot.trn_boot import _ntff_profile_via_ctypes
    except Exception:
        return
    hook = None
    try:
        hook = _ntff_profile_via_ctypes("/opt/axon/libaxon_pjrt.so")
    except Exception:
        hook = None
    mod = types.ModuleType("antenv.axon_hooks")
    mod._hook = hook
    mod.get_axon_ntff_profile_hook = lambda: mod._hook
    mod.set_axon_ntff_profile_hook = lambda h: setattr(mod, "_hook", h)
    sys.modules["antenv.axon_hooks"] = mod


def _run(inputs, trace=False):
    from concourse import bass_utils

    if trace:
        _ensure_ntff_hook()

    nc = _get_nc()
    f16 = np.float16
    query = np.asarray(inputs["query"], np.float32)
    key = np.asarray(inputs["key"], np.float32)
    value = np.asarray(inputs["value"], np.float32)
    Wq = np.asarray(inputs["Wq"], np.float32)
    Wk = np.asarray(inputs["Wk"], np.float32)
    Wv = np.asarray(inputs["Wv"], np.float32)
    Wo = np.asarray(inputs["Wo"], np.float32)
    bqv = np.asarray(inputs["bq"], np.float32)
    bkv = np.asarray(inputs["bk"], np.float32)
    bvv = np.asarray(inputs["bv"], np.float32)
    bo = np.asarray(inputs["bo"], np.float32)

    xqT = np.ascontiguousarray(query.reshape(NTOK, D).T.astype(f16))
    xkT = np.ascontiguousarray(key.reshape(NTOK, D).T.astype(f16))
    xvT = np.ascontiguousarray(value.reshape(NTOK, D).T.astype(f16))

    def pack_w(Wc):
        return np.ascontiguousarray(
            Wc.reshape(8, 128, 128).transpose(1, 0, 2).reshape(128, D).astype(f16)
        )

    ident_np = np.zeros((128, 64), np.float32)
    ident_np[np.arange(64), np.arange(64)] = 1.0
    ident_np[64 + np.arange(64), np.arange(64)] = 1.0
    consts = {
        "c_ident": np.ascontiguousarray(ident_np.astype(f16)),
        "c_ones64": np.ones((1, 64), f16),
    }
    in_maps = []
    for c in range(N_CORES):
        sl = slice(c * 128, (c + 1) * 128)
        in_maps.append(
            {
                **consts,
                "xq": xqT,
                "xk": xkT,
                "xv": xvT,
                "wq": pack_w(Wq[:, sl]),
                "wk": pack_w(Wk[:, sl]),
                "wv": pack_w(Wv[:, sl]),
                "wo": np.ascontiguousarray(Wo[sl, :].astype(f16)),
                "bq": np.ascontiguousarray(bqv[sl].reshape(128, 1).astype(f16)),
                "bk": np.ascontiguousarray(bkv[sl].reshape(128, 1).astype(f16)),
                "bv": np.ascontiguousarray(bvv[sl].reshape(128, 1).astype(f16)),
            }
        )

    res = bass_utils.run_bass_kernel_spmd(
        nc, in_maps, core_ids=list(range(N_CORES)), trace=trace
    )
    outT = np.zeros((D, NTOK), np.float32)
    for c in range(N_CORES):
        outT += np.asarray(res.results[c]["pout"], np.float32)
    out = (outT.T + bo.astype(np.float32)).astype(np.float32)
    return out.reshape(B, S, D), res


def kernel(**inputs):
    out, _ = _run(inputs, trace=False)
    return out
